# revision 23
# baseline (speedup 1.0000x reference)
"""GATv2 model kernel for Trainium2 (Bass/Tile), data-parallel over batch on 8 cores.

Model (per graph b): input MLP -> 4 GATv2 layers (dense N^2 attention with
edge features) -> sum-pool -> linear head.  B=16, N=128, HID=128, H=8, C=16.

V2 strategy ("category-structured"): the N x N edge-category matrix cat[i,j]
has only 20 distinct values (pos(i)*4+pos(j) off-diagonal, 16+pos(i) on the
diagonal, pos = node//32).  So e_feat @ We[l] collapses to 20 columns EW and
the GATv2 message m[i,j,:] = xl[i] + xr[j] + e_cat is an outer sum of two
[128,128] tiles per 32-target-column block:

  - xl'_q = xlT + EW[:, p*4+q] built per target-orbit q (4 tiny DVE stts)
  - m_q[hc,(j,i)] assembled by ONE DVE scalar_tensor_tensor per q with both
    operands free-dim-broadcast (no giant PE matmuls, no e_feat in HBM)
  - leaky-relu applied in one fused op (DVE max(0.2m, m) / ACT Prelu split;
    Prelu shares the natural_log_exp activation table with Exp/Ln/Square so
    the ACT engine never reloads tables)
  - scores per target j via tiny f16 matmuls (lhsT=lrelu(m)_j, rhs=att
    block-diag); PE issues these back-to-back at ~25ns
  - the diagonal uses categories 16+p: fixed EXACTLY via a rank-1 correction
    po += (exp(s_diag_true) - exp(s_diag_wrong)) * [xl|1] on the augmented
    aggregation PSUM (all partition-aligned; diag of m_q read with a
    stride-129 free-dim AP)
  - softmax normalizer from the augmented ones column as in V1; LayerNorm
    rsqrt via exp(-0.5*ln(var+eps)) keeps every ACT function in ONE table.
"""

import numpy as np
from contextlib import ExitStack

import concourse.bacc as bacc
import concourse.bass as bass
import concourse.tile as tile
from concourse import mybir
from concourse.masks import make_identity

F32 = mybir.dt.float32
F16 = mybir.dt.float16
AF = mybir.ActivationFunctionType
OP = mybir.AluOpType
AX = mybir.AxisListType

B, N, HID, H, C, L = 16, 128, 128, 8, 16, 4
NCORES = 8
BL = B // NCORES          # graphs per core
NEG = 0.2                 # leaky relu slope
NQ = 4                    # orbits
JQ = N // NQ              # 32 targets per orbit block
EPS = 1e-5
AUG = 17                  # head dim + 1 (softmax normalizer column)
NCAT = NQ * NQ + NQ       # 20 edge categories

# LN vector slots in the replicated-params tile
LN1G, LN1B, LN2G, LN2B = 0, 1, 2, 3
LNG0, LNB0 = 4, 8

import os
ACT_COLS = int(os.environ.get("KERNEL_ACT_COLS", "4096"))  # lrelu cols on ACT per q


def _ln_free(nc, wp, sp, pin, g_ap, b_ap, out_ap, uid, zb, epsb):
    """LayerNorm along the free dim of pin [128, D] -> out_ap (SBUF)."""
    D = pin.shape[-1]
    mu = sp.tile([128, 1], F32, tag=f"mu{uid}")
    nc.vector.tensor_reduce(mu, pin, axis=AX.X, op=OP.add)
    nc.vector.tensor_scalar_mul(mu, mu, 1.0 / D)
    t = wp.tile([128, D], F32, tag=f"lnc{uid}")
    nc.vector.tensor_scalar_sub(t, pin, mu)
    sq = wp.tile([128, D], F32, tag=f"lnsq{uid}")
    vs = sp.tile([128, 1], F32, tag=f"vs{uid}")
    nc.scalar.activation(sq, t, AF.Square, bias=zb, accum_out=vs)
    lv = sp.tile([128, 1], F32, tag=f"lv{uid}")
    nc.scalar.activation(lv, vs, AF.Ln, scale=1.0 / D, bias=epsb)
    rstd = sp.tile([128, 1], F32, tag=f"rstd{uid}")
    nc.scalar.activation(rstd, lv, AF.Exp, scale=-0.5, bias=zb)
    nc.vector.scalar_tensor_tensor(out_ap, t, rstd, g_ap, op0=OP.mult, op1=OP.mult)
    nc.vector.tensor_add(out_ap, out_ap, b_ap)


class _Bacc(bacc.Bacc):
    # Prefer the one activation table containing Exp+Ln+Square+Identity+Prelu
    # so the ACT engine never reloads tables mid-kernel (1283ns each).
    def insert_act_table_loads(self):
        from concourse.hw_specs import get_activation_tables
        import bass_rust as _br
        has = any(isinstance(i, mybir.InstActivation)
                  for b in self.main_func.blocks for i in b.instructions)
        if not has:
            return
        tables = [
            (name, s if name == "natural_log_exp_and_others" else set())
            for name, s in get_activation_tables(self.m.arch).items()
        ]
        _br.insert_act_table_loads(self, tables)


def build_nc():
    nc = _Bacc("TRN2", target_bir_lowering=False, debug=False)

    xT = nc.dram_tensor("xT", [2, BL * N], F16, kind="ExternalInput")
    wl = nc.dram_tensor("wl", [HID, L * HID], F16, kind="ExternalInput")
    wr = nc.dram_tensor("wr", [HID, L * HID], F16, kind="ExternalInput")
    pw = nc.dram_tensor("pw", [HID, L * HID], F16, kind="ExternalInput")
    ewt = nc.dram_tensor("ewt", [NCAT, L * HID], F16, kind="ExternalInput")
    catmask = nc.dram_tensor("catmask", [NCAT, 5 * N], F16, kind="ExternalInput")
    ab08 = nc.dram_tensor("ab08", [HID, L * H], F16, kind="ExternalInput")
    ab02 = nc.dram_tensor("ab02", [HID, L * H], F16, kind="ExternalInput")
    ab02rep = nc.dram_tensor("ab02rep", [HID, L * JQ * H], F16, kind="ExternalInput")
    blT = nc.dram_tensor("blT", [HID, L], F32, kind="ExternalInput")
    brow = nc.dram_tensor("brow", [1, 10 * HID], F16, kind="ExternalInput")
    mw1 = nc.dram_tensor("mw1", [2, HID], F16, kind="ExternalInput")
    mw2 = nc.dram_tensor("mw2", [HID, HID], F16, kind="ExternalInput")
    lnr = nc.dram_tensor("lnr", [HID, 12 * HID], F16, kind="ExternalInput")
    ow = nc.dram_tensor("ow", [HID, 1], F32, kind="ExternalInput")
    ob = nc.dram_tensor("ob", [1, 1], F32, kind="ExternalInput")
    out = nc.dram_tensor("out", [BL, 1], F32, kind="ExternalOutput")

    with tile.TileContext(nc) as tc, ExitStack() as ctx:
        cp = ctx.enter_context(tc.tile_pool(name="const", bufs=1))
        pp = ctx.enter_context(tc.tile_pool(name="perb", bufs=1))
        hp = ctx.enter_context(tc.tile_pool(name="hpool", bufs=2))
        wp = ctx.enter_context(tc.tile_pool(name="work", bufs=3))
        sp = ctx.enter_context(tc.tile_pool(name="small", bufs=4))
        mb = ctx.enter_context(tc.tile_pool(name="mb", bufs=2))
        pt = ctx.enter_context(tc.tile_pool(name="pt", bufs=1, space="PSUM"))
        pg = ctx.enter_context(tc.tile_pool(name="pg", bufs=1, space="PSUM"))
        pm = ctx.enter_context(tc.tile_pool(name="pm", bufs=1, space="PSUM"))
        pq = ctx.enter_context(tc.tile_pool(name="pq", bufs=2, space="PSUM"))
        ps = ctx.enter_context(tc.tile_pool(name="ps", bufs=2, space="PSUM"))
        pDO = ctx.enter_context(tc.tile_pool(name="pDO", bufs=1, space="PSUM"))

        # ---- load constants ----
        def load(dram, shape, name, dt=F16):
            t = cp.tile(shape, dt, tag=name)
            nc.sync.dma_start(t[:], dram[:])
            return t

        xT_s = load(xT, [2, BL * N], "xT")
        mw1_s = load(mw1, [2, HID], "mw1")
        brow_s = load(brow, [1, 10 * HID], "brow")
        lnr_s = load(lnr, [HID, 12 * HID], "lnr")
        mw2_s = load(mw2, [HID, HID], "mw2")
        wl_s = load(wl, [HID, L * HID], "wl")
        wr_s = load(wr, [HID, L * HID], "wr")
        ewt_s = load(ewt, [NCAT, L * HID], "ewt")
        cm_s = load(catmask, [NCAT, 5 * N], "cm")
        ab08_s = load(ab08, [HID, L * H], "ab08")
        ab02_s = load(ab02, [HID, L * H], "ab02")
        ab02r_s = load(ab02rep, [HID, L * JQ * H], "ab02r")
        blT_s = load(blT, [HID, L], "blT", F32)
        pw_s = load(pw, [HID, L * HID], "pw")
        ow_s = load(ow, [HID, 1], "ow", F32)
        ob_s = load(ob, [1, 1], "ob", F32)

        ident = cp.tile([128, 128], F32, tag="ident")
        make_identity(nc, ident[:])
        ident16 = cp.tile([128, 128], F16, tag="ident16")
        nc.vector.tensor_copy(ident16, ident)
        ones_r = cp.tile([1, N], F16, tag="ones_r")
        nc.gpsimd.memset(ones_r[:], 1.0)
        ones640 = cp.tile([1, 5 * N], F16, tag="ones640")
        nc.gpsimd.memset(ones640[:], 1.0)
        ones_c = cp.tile([128, 1], F16, tag="ones_c")
        nc.gpsimd.memset(ones_c[:], 1.0)
        zb = cp.tile([128, 1], F32, tag="zb")
        nc.gpsimd.memset(zb[:], 0.0)
        epsb = cp.tile([128, 1], F32, tag="epsb")
        nc.gpsimd.memset(epsb[:], EPS)

        # per-graph persistent: augmented [xl | 1] (ones col survives memset)
        xla = pp.tile([128, BL * H * AUG], F16, tag="xla")
        nc.gpsimd.memset(xla[:], 1.0)

        def lnv(i):  # replicated LN vector slice [128, 128]
            return lnr_s[:, i * HID:(i + 1) * HID]

        h_cur = [None] * BL

        # ======== input MLP ========
        for b in range(BL):
            p1 = pg.tile([128, HID], F32, tag="pg")
            nc.tensor.matmul(p1, xT_s[:, b * N:(b + 1) * N], mw1_s[:], start=True, stop=False)
            nc.tensor.matmul(p1, ones_r[:], brow_s[:, 0:HID], start=False, stop=True)
            h1 = wp.tile([128, HID], F32, tag="h1")
            _ln_free(nc, wp, sp, p1[:], lnv(LN1G), lnv(LN1B), h1[:], "a", zb, epsb)
            h1r = wp.tile([128, HID], F16, tag="h1r")
            nc.vector.tensor_scalar_max(h1r, h1, 0.0)
            ptr = pt.tile([128, 128], F16, tag="ptr")
            nc.tensor.transpose(ptr, h1r[:], ident16[:])
            h1T = wp.tile([128, HID], F16, tag="h1T")
            nc.vector.tensor_copy(h1T, ptr)
            p2 = pg.tile([128, HID], F32, tag="pg")
            nc.tensor.matmul(p2, h1T[:], mw2_s[:], start=True, stop=False)
            nc.tensor.matmul(p2, ones_r[:], brow_s[:, HID:2 * HID], start=False, stop=True)
            hb = hp.tile([128, HID], F16, tag=f"h{b}")
            _ln_free(nc, wp, sp, p2[:], lnv(LN2G), lnv(LN2B), hb[:], "b", zb, epsb)
            h_cur[b] = hb

        # ======== GATv2 layers ========
        # Scores use lrelu = 0.2*m + 0.8*relu(m): the 0.2-linear xl/e part is a
        # per-q broadcast matmul into psb, the xr part is per-(j,h)-constant and
        # dropped (softmax-invariant; dropped consistently in the diag terms).
        # The two graphs are emitted phase-interleaved for pipeline overlap.
        PE_Q = int(os.environ.get("KERNEL_PE_Q", "1"))       # q-blocks on PE
        DVE_CHUNKS = int(os.environ.get("KERNEL_DVE_CHUNKS", "4"))  # PSUM relu chunks on DVE (of 8)

        def prep(l, b):
            wls = wl_s[:, l * HID:(l + 1) * HID]
            wrs = wr_s[:, l * HID:(l + 1) * HID]
            ewtl = ewt_s[:, l * HID:(l + 1) * HID]
            blrow = brow_s[:, (2 + l) * HID:(3 + l) * HID]
            hb = h_cur[b]
            xlab = xla[:, b * H * AUG:(b + 1) * H * AUG]
            ptr = pt.tile([128, 128], F16, tag="ptr")
            nc.tensor.transpose(ptr, hb[:], ident16[:])
            hT16 = wp.tile([128, HID], F16, tag="hT16")
            nc.scalar.activation(hT16, ptr, AF.Identity, bias=zb)

            pxq = pm.tile([128, 4 * 128], F32, tag="pxq")
            nc.tensor.matmul(pxq.rearrange("k (a i) -> k a i", a=4),
                             wls,
                             hT16.rearrange("k (o i) -> k o i", o=1)
                             .broadcast_to((128, 4, 128)), start=True, stop=False)
            nc.tensor.matmul(pxq, ewtl[0:NCAT, :], cm_s[0:NCAT, 0:4 * N],
                             start=False, stop=False)
            nc.tensor.matmul(pxq, blrow, ones640[:, 0:4 * N], start=False, stop=True)
            xlq = wp.tile([128, 4 * 128], F16, tag="xlq")
            nc.scalar.activation(xlq, pxq, AF.Identity, bias=zb)

            pxd = pg.tile([128, HID], F32, tag="pg")
            nc.tensor.matmul(pxd, wls, hT16[:], start=True, stop=False)
            nc.tensor.matmul(pxd, ewtl[0:NCAT, :], cm_s[0:NCAT, 4 * N:5 * N],
                             start=False, stop=False)
            nc.tensor.matmul(pxd, blrow, ones_r[:], start=False, stop=True)
            xld16 = wp.tile([128, HID], F16, tag="xld16")
            nc.scalar.activation(xld16, pxd, AF.Identity, bias=zb)

            pxr = pg.tile([128, HID], F32, tag="pg")
            nc.tensor.matmul(pxr, wrs, hT16[:], start=True, stop=True)
            xrT = wp.tile([128, HID], F16, tag="xrT")
            nc.scalar.activation(xrT, pxr, AF.Identity, bias=zb)
            pxr2 = pg.tile([128, HID], F32, tag="pg")
            nc.tensor.matmul(pxr2, hT16[:], wrs, start=True, stop=True)
            xrn = wp.tile([128, HID], F16, tag="xrn")
            nc.scalar.activation(xrn, pxr2, AF.Identity, bias=zb)

            pxn = pg.tile([128, HID], F32, tag="pg")
            nc.tensor.matmul(pxn, hT16[:], wls, start=True, stop=False)
            nc.tensor.matmul(pxn, ones_r[:], blrow, start=False, stop=True)
            nc.scalar.activation(
                xlab.rearrange("i (h q) -> i h q", q=AUG)[:, :, 0:C],
                pxn.rearrange("i (h c) -> i h c", c=C), AF.Identity, bias=zb)

            md = wp.tile([128, 128], F16, tag="md")
            nc.vector.tensor_add(md, xld16, xrT)
            mdl = wp.tile([128, 128], F16, tag="mdl")
            nc.vector.tensor_scalar_max(mdl, md, 0.0)
            mab = mb.tile([128, N * 128], F16, tag="mab")
            return dict(xlq=xlq, xld16=xld16, xrT=xrT, xrn=xrn, mdl=mdl,
                        mab=mab, xlab=xlab, psbh=[None, None], es=None)

        def mblock(l, b, q, d):
            ab08l = ab08_s[:, l * H:(l + 1) * H]
            ab02rl = ab02r_s[:, l * JQ * H:(l + 1) * JQ * H]
            xlq, xrT, xrn, mab = d["xlq"], d["xrT"], d["xrn"], d["mab"]
            half = q // 2
            if q % 2 == 0:
                psbh_t = ps.tile([128, JQ * 2 * H], F32, tag="psbh")
                d["psbh"][half] = psbh_t
            psbh = d["psbh"][half]
            pcols = (q % 2) * JQ * H
            nc.tensor.matmul(psbh[:, pcols:pcols + JQ * H],
                             xlq[:, q * 128:(q + 1) * 128],
                             ab02rl, start=True, stop=False)
            if q >= NQ - PE_Q:
                for c in range(8):
                    j0 = q * JQ + c * 4
                    pmq = pq.tile([128, 512], F32, tag="pmq")
                    nc.tensor.matmul(
                        pmq.rearrange("k (j i) -> k j i", j=4),
                        ident16[:],
                        xlq[:, q * 128:(q + 1) * 128]
                        .rearrange("k (o i) -> k o i", o=1)
                        .broadcast_to((128, 4, 128)), start=True, stop=False)
                    for jj in range(4):
                        nc.tensor.matmul(
                            pmq[:, jj * 128:(jj + 1) * 128],
                            xrn[:],
                            ident16[:, j0 + jj:j0 + jj + 1]
                            .broadcast_to((128, 128)),
                            start=False, stop=jj == 3,
                            skip_group_check=jj != 3)
                    dst = mab[:, j0 * 128:(j0 + 4) * 128]
                    if c < 8 - DVE_CHUNKS:
                        nc.scalar.activation(dst, pmq, AF.Relu, bias=zb)
                    else:
                        nc.vector.tensor_scalar_max(dst, pmq, 0.0)
            else:
                for hh in range(2):
                    m_q = mb.tile([128, JQ * 64], F16, tag="m_q")
                    j0 = q * JQ + hh * 16
                    nc.vector.scalar_tensor_tensor(
                        m_q.rearrange("k (j i) -> k j i", j=16),
                        xlq[:, q * 128:(q + 1) * 128]
                        .rearrange("k (o i) -> k o i", o=1)
                        .broadcast_to((128, 16, 128)),
                        0.0,
                        xrT[:, j0:j0 + 16]
                        .rearrange("k (j o) -> k j o", o=1)
                        .broadcast_to((128, 16, 128)),
                        op0=OP.add, op1=OP.add)
                    nc.scalar.activation(mab[:, j0 * 128:(j0 + 16) * 128],
                                         m_q[:], AF.Relu, bias=zb)
            for jj in range(JQ):
                j = q * JQ + jj
                last = jj == JQ - 1
                nc.tensor.matmul(psbh[:, pcols + jj * H:pcols + (jj + 1) * H],
                                 mab[:, j * 128:(j + 1) * 128], ab08l,
                                 start=False, stop=last,
                                 skip_group_check=not last)
            if q % 2 == 1:
                if d["es"] is None:
                    es_t = wp.tile([128, N * H], F16, tag="es")
                    d["es"] = es_t
                nc.scalar.activation(d["es"][:, half * JQ * 2 * H:(half + 1) * JQ * 2 * H],
                                     psbh[:], AF.Exp, bias=zb)

        def tail(l, b, d):
            ab08l = ab08_s[:, l * H:(l + 1) * H]
            ab02l = ab02_s[:, l * H:(l + 1) * H]
            pws = pw_s[:, l * HID:(l + 1) * HID]
            xlq, mab, mdl, xlab, esb = d["xlq"], d["mab"], d["mdl"], d["xlab"], d["es"]
            pdo = pDO.tile([128, 160], F32, tag="pdo")
            pd = pdo[:, 144:160]
            po = pdo[:, 0:H * AUG]
            nc.tensor.matmul(pd[:, 0:8], mdl[:], ab08l, start=True, stop=False)
            nc.tensor.matmul(pd[:, 0:8], d["xld16"][:], ab02l, start=False, stop=True)
            dsl = mab[:, 0:(N - 1) * 129 + 1:129]
            nc.tensor.matmul(pd[:, 8:16], dsl, ab08l, start=True, stop=False)
            dlin = wp.tile([128, 128], F16, tag="dlin")
            nc.scalar.activation(
                dlin.rearrange("k (q r) -> k q r", q=NQ),
                xlq.rearrange("k (a b) -> k a b", a=16)[:, 0:16:5, :],
                AF.Identity, bias=zb)
            nc.tensor.matmul(pd[:, 8:16], dlin[:], ab02l, start=False, stop=True)
            ed = sp.tile([128, 16], F32, tag="ed")
            nc.scalar.activation(ed, pd, AF.Exp, bias=zb)
            Dt = sp.tile([128, 8], F32, tag="Dt")
            nc.vector.tensor_sub(Dt, ed[:, 0:8], ed[:, 8:16])

            es3 = esb.rearrange("i (j h) -> i j h", h=H)
            for h in range(H):
                nc.tensor.matmul(po[:, h * AUG:(h + 1) * AUG], es3[:, :, h],
                                 xlab[:, h * AUG:(h + 1) * AUG], start=True, stop=True)
            tcor = wp.tile([128, H * AUG], F32, tag="tcor")
            nc.vector.scalar_tensor_tensor(
                tcor.rearrange("j (h q) -> j h q", q=AUG),
                xlab.rearrange("j (h q) -> j h q", q=AUG), 1.0,
                Dt.rearrange("j (h o) -> j h o", o=1).broadcast_to((128, H, AUG)),
                op0=OP.mult, op1=OP.mult)
            nc.vector.tensor_add(po, po, tcor)

            po3 = po.rearrange("j (h q) -> j h q", q=AUG)
            rz = sp.tile([128, H], F32, tag="rz")
            nc.vector.reciprocal(rz.rearrange("j (h o) -> j h o", o=1),
                                 po3[:, :, C:C + 1])
            o16 = wp.tile([128, HID], F16, tag="o16")
            nc.vector.scalar_tensor_tensor(
                o16.rearrange("j (h c) -> j h c", c=C), po3[:, :, 0:C], 1.0,
                rz.rearrange("j (h o) -> j h o", o=1).broadcast_to((128, H, C)),
                op0=OP.mult, op1=OP.mult)

            pto = pt.tile([128, 128], F16, tag="ptr")
            nc.tensor.transpose(pto, o16[:], ident16[:])
            oT = wp.tile([128, HID], F16, tag="oT")
            nc.scalar.activation(oT, pto, AF.Identity, bias=zb)
            ppj = pg.tile([128, HID], F32, tag="pg")
            nc.tensor.matmul(ppj, oT[:], pws, start=True, stop=False)
            nc.tensor.matmul(ppj, ones_r[:], brow_s[:, (6 + l) * HID:(7 + l) * HID],
                             start=False, stop=True)
            lno = wp.tile([128, HID], F32, tag="lno")
            _ln_free(nc, wp, sp, ppj[:], lnv(LNG0 + l), lnv(LNB0 + l), lno[:], "c", zb, epsb)
            hn = hp.tile([128, HID], F16, tag=f"h{b}")
            nc.vector.scalar_tensor_tensor(hn, lno, 0.0, h_cur[b],
                                           op0=OP.max, op1=OP.add)
            h_cur[b] = hn

        # software-pipeline the two graphs half a layer out of phase: one
        # graph's m-blocks overlap the other's tail+prep dependency chains
        ds = [None, None]
        ds[0] = prep(0, 0)
        for q in range(NQ):
            mblock(0, 0, q, ds[0])
        ds[1] = prep(0, 1)
        for l in range(L):
            for q in range(NQ):
                mblock(l, 1, q, ds[1])
            tail(l, 0, ds[0])
            if l + 1 < L:
                ds[0] = prep(l + 1, 0)
                for q in range(NQ):
                    mblock(l + 1, 0, q, ds[0])
            tail(l, 1, ds[1])
            if l + 1 < L:
                ds[1] = prep(l + 1, 1)

        # ======== pooling + head ========
        for b in range(BL):
            pa = pg.tile([128, 1], F32, tag="pg")
            nc.tensor.matmul(pa, h_cur[b][:], ones_c[:], start=True, stop=True)
            hagg = sp.tile([128, 1], F32, tag="hagg")
            nc.vector.tensor_copy(hagg, pa)
            pr = pg.tile([1, 1], F32, tag="pg")
            nc.tensor.matmul(pr, hagg[:], ow_s[:], start=True, stop=True)
            res = sp.tile([1, 1], F32, tag="res")
            nc.scalar.activation(res, pr, AF.Identity, bias=ob_s[0:1, 0:1])
            nc.sync.dma_start(out[b:b + 1, :], res[:])

    nc.compile()
    return nc


def pack_inputs(inputs):
    """Full model inputs -> per-core in_maps (host-side shard + re-layout)."""
    f = {k: np.asarray(v, dtype=np.float32) if k != "cat" else np.asarray(v)
         for k, v in inputs.items()}
    att = f["att"]
    abk = np.zeros((HID, L * H), np.float32)
    for l in range(L):
        for h in range(H):
            abk[h * C:(h + 1) * C, l * H + h] = att[l, h]

    # category mask [20, 5*128]: block q (4 off-diag target orbits) + diag block
    pos = np.arange(N) // (N // NQ)
    catmask = np.zeros((NCAT, 5 * N), np.float32)
    for q in range(NQ):
        catmask[pos * NQ + q, q * N + np.arange(N)] = 1.0
    catmask[NQ * NQ + pos, NQ * N + np.arange(N)] = 1.0

    pb_eff = np.stack([f["cb"][l] @ f["pW"][l] + f["pb"][l] for l in range(L)])

    # EW: per layer the 20 category embeddings transformed by We [20, HID]
    ewt = np.zeros((NCAT, L * HID), np.float32)
    for l in range(L):
        ewt[:, l * HID:(l + 1) * HID] = f["emb"] @ f["We"][l]

    lnvecs = [f["ln1_g"], f["ln1_b"], f["ln2_g"], f["ln2_b"],
              *[f["lng"][l] for l in range(L)], *[f["lnb"][l] for l in range(L)]]
    lnr = np.ascontiguousarray(
        np.broadcast_to(np.concatenate(lnvecs)[None, :], (HID, 12 * HID))
    ).astype(np.float16)

    def stackw(w):  # [L, k, hc] -> [k, L*hc] so sbuf slice l is W[l][k, hc]
        return np.ascontiguousarray(w.transpose(1, 0, 2).reshape(HID, L * HID))

    shared = {
        "wl": stackw(f["Wl"]).astype(np.float16),
        "wr": stackw(f["Wr"]).astype(np.float16),
        "pw": stackw(f["pW"]).astype(np.float16),
        "ewt": ewt.astype(np.float16),
        "catmask": catmask.astype(np.float16),
        "ab08": (0.8 * abk).astype(np.float16),
        "ab02": (0.2 * abk).astype(np.float16),
        "ab02rep": np.ascontiguousarray(
            np.tile((0.2 * abk).reshape(HID, L, 1, H), (1, 1, JQ, 1))
            .reshape(HID, L * JQ * H)).astype(np.float16),
        "blT": np.ascontiguousarray(f["bl"].T),
        "brow": np.concatenate([f["mlp_b1"], f["mlp_b2"],
                                f["bl"].ravel(), pb_eff.ravel()])
        .reshape(1, 10 * HID).astype(np.float16),
        "mw1": f["mlp_w1"].astype(np.float16),
        "mw2": f["mlp_w2"].astype(np.float16),
        "lnr": lnr, "ow": f["out_w"].reshape(HID, 1),
        "ob": f["out_b"].reshape(1, 1),
    }
    in_maps = []
    for c in range(NCORES):
        xTc = np.ascontiguousarray(
            f["x"][c * BL:(c + 1) * BL].transpose(2, 0, 1)).reshape(2, BL * N)
        m = dict(shared)
        m["xT"] = xTc.astype(np.float16)
        in_maps.append(m)
    return in_maps


_NC = None
LAST_EXEC_NS = None


def kernel(**inputs) -> np.ndarray:
    global _NC, LAST_EXEC_NS
    from concourse.bass_utils import run_bass_kernel_spmd
    if _NC is None:
        _NC = build_nc()
    import os
    in_maps = pack_inputs(inputs)
    trace = bool(os.environ.get("KERNEL_TRACE"))
    kw = {}
    td = os.environ.get("KERNEL_TRACE_DIR")
    if td:
        os.makedirs(td, exist_ok=True)
        kw["tmpdir"] = td
    r = run_bass_kernel_spmd(_NC, in_maps, core_ids=list(range(NCORES)),
                             trace=trace, **kw)
    LAST_EXEC_NS = r.exec_time_ns
    out = np.concatenate([r.results[c]["out"] for c in range(NCORES)], axis=0)
    return out.astype(np.float32)


# revision 24
# speedup vs baseline: 1.0611x; 1.0611x over previous
"""GATv2 model kernel for Trainium2 (Bass/Tile), data-parallel over batch on 8 cores.

Model (per graph b): input MLP -> 4 GATv2 layers (dense N^2 attention with
edge features) -> sum-pool -> linear head.  B=16, N=128, HID=128, H=8, C=16.

V2 strategy ("category-structured"): the N x N edge-category matrix cat[i,j]
has only 20 distinct values (pos(i)*4+pos(j) off-diagonal, 16+pos(i) on the
diagonal, pos = node//32).  So e_feat @ We[l] collapses to 20 columns EW and
the GATv2 message m[i,j,:] = xl[i] + xr[j] + e_cat is an outer sum of two
[128,128] tiles per 32-target-column block:

  - xl'_q = xlT + EW[:, p*4+q] built per target-orbit q (4 tiny DVE stts)
  - m_q[hc,(j,i)] assembled by ONE DVE scalar_tensor_tensor per q with both
    operands free-dim-broadcast (no giant PE matmuls, no e_feat in HBM)
  - leaky-relu applied in one fused op (DVE max(0.2m, m) / ACT Prelu split;
    Prelu shares the natural_log_exp activation table with Exp/Ln/Square so
    the ACT engine never reloads tables)
  - scores per target j via tiny f16 matmuls (lhsT=lrelu(m)_j, rhs=att
    block-diag); PE issues these back-to-back at ~25ns
  - the diagonal uses categories 16+p: fixed EXACTLY via a rank-1 correction
    po += (exp(s_diag_true) - exp(s_diag_wrong)) * [xl|1] on the augmented
    aggregation PSUM (all partition-aligned; diag of m_q read with a
    stride-129 free-dim AP)
  - softmax normalizer from the augmented ones column as in V1; LayerNorm
    rsqrt via exp(-0.5*ln(var+eps)) keeps every ACT function in ONE table.
"""

import numpy as np
from contextlib import ExitStack

import concourse.bacc as bacc
import concourse.bass as bass
import concourse.tile as tile
from concourse import mybir
from concourse.masks import make_identity

F32 = mybir.dt.float32
F16 = mybir.dt.float16
AF = mybir.ActivationFunctionType
OP = mybir.AluOpType
AX = mybir.AxisListType

B, N, HID, H, C, L = 16, 128, 128, 8, 16, 4
NCORES = 8
BL = B // NCORES          # graphs per core
NEG = 0.2                 # leaky relu slope
NQ = 4                    # orbits
JQ = N // NQ              # 32 targets per orbit block
EPS = 1e-5
AUG = 17                  # head dim + 1 (softmax normalizer column)
NCAT = NQ * NQ + NQ       # 20 edge categories

# LN vector slots in the replicated-params tile
LN1G, LN1B, LN2G, LN2B = 0, 1, 2, 3
LNG0, LNB0 = 4, 8

import os
ACT_COLS = int(os.environ.get("KERNEL_ACT_COLS", "4096"))  # lrelu cols on ACT per q


def _ln_free(nc, wp, sp, pin, g_ap, b_ap, out_ap, uid, zb, epsb):
    """LayerNorm along the free dim of pin [128, D] -> out_ap (SBUF)."""
    D = pin.shape[-1]
    mu = sp.tile([128, 1], F32, tag=f"mu{uid}")
    nc.vector.tensor_reduce(mu, pin, axis=AX.X, op=OP.add)
    nc.vector.tensor_scalar_mul(mu, mu, 1.0 / D)
    t = wp.tile([128, D], F32, tag=f"lnc{uid}")
    nc.vector.tensor_scalar_sub(t, pin, mu)
    sq = wp.tile([128, D], F32, tag=f"lnsq{uid}")
    vs = sp.tile([128, 1], F32, tag=f"vs{uid}")
    nc.scalar.activation(sq, t, AF.Square, bias=zb, accum_out=vs)
    lv = sp.tile([128, 1], F32, tag=f"lv{uid}")
    nc.scalar.activation(lv, vs, AF.Ln, scale=1.0 / D, bias=epsb)
    rstd = sp.tile([128, 1], F32, tag=f"rstd{uid}")
    nc.scalar.activation(rstd, lv, AF.Exp, scale=-0.5, bias=zb)
    nc.vector.scalar_tensor_tensor(out_ap, t, rstd, g_ap, op0=OP.mult, op1=OP.mult)
    nc.vector.tensor_add(out_ap, out_ap, b_ap)


class _Bacc(bacc.Bacc):
    # Prefer the one activation table containing Exp+Ln+Square+Identity+Prelu
    # so the ACT engine never reloads tables mid-kernel (1283ns each).
    def insert_act_table_loads(self):
        from concourse.hw_specs import get_activation_tables
        import bass_rust as _br
        has = any(isinstance(i, mybir.InstActivation)
                  for b in self.main_func.blocks for i in b.instructions)
        if not has:
            return
        tables = [
            (name, s if name == "natural_log_exp_and_others" else set())
            for name, s in get_activation_tables(self.m.arch).items()
        ]
        _br.insert_act_table_loads(self, tables)


def build_nc():
    nc = _Bacc("TRN2", target_bir_lowering=False, debug=False)

    xT = nc.dram_tensor("xT", [2, BL * N], F16, kind="ExternalInput")
    wl = nc.dram_tensor("wl", [HID, L * HID], F16, kind="ExternalInput")
    wr = nc.dram_tensor("wr", [HID, L * HID], F16, kind="ExternalInput")
    pw = nc.dram_tensor("pw", [HID, L * HID], F16, kind="ExternalInput")
    ewt = nc.dram_tensor("ewt", [NCAT, L * HID], F16, kind="ExternalInput")
    catmask = nc.dram_tensor("catmask", [NCAT, 5 * N], F16, kind="ExternalInput")
    ab08 = nc.dram_tensor("ab08", [HID, L * H], F16, kind="ExternalInput")
    ab02 = nc.dram_tensor("ab02", [HID, L * H], F16, kind="ExternalInput")
    ab02rep = nc.dram_tensor("ab02rep", [HID, L * JQ * H], F16, kind="ExternalInput")
    blT = nc.dram_tensor("blT", [HID, L], F32, kind="ExternalInput")
    brow = nc.dram_tensor("brow", [1, 10 * HID], F16, kind="ExternalInput")
    mw1 = nc.dram_tensor("mw1", [2, HID], F16, kind="ExternalInput")
    mw2 = nc.dram_tensor("mw2", [HID, HID], F16, kind="ExternalInput")
    lnr = nc.dram_tensor("lnr", [HID, 12 * HID], F16, kind="ExternalInput")
    ow = nc.dram_tensor("ow", [HID, 1], F32, kind="ExternalInput")
    ob = nc.dram_tensor("ob", [1, 1], F32, kind="ExternalInput")
    out = nc.dram_tensor("out", [BL, 1], F32, kind="ExternalOutput")

    with tile.TileContext(nc) as tc, ExitStack() as ctx:
        cp = ctx.enter_context(tc.tile_pool(name="const", bufs=1))
        pp = ctx.enter_context(tc.tile_pool(name="perb", bufs=1))
        hp = ctx.enter_context(tc.tile_pool(name="hpool", bufs=2))
        wp = ctx.enter_context(tc.tile_pool(name="work", bufs=3))
        sp = ctx.enter_context(tc.tile_pool(name="small", bufs=4))
        mb = ctx.enter_context(tc.tile_pool(name="mb", bufs=2))
        pt = ctx.enter_context(tc.tile_pool(name="pt", bufs=1, space="PSUM"))
        pg = ctx.enter_context(tc.tile_pool(name="pg", bufs=2, space="PSUM"))
        pm = ctx.enter_context(tc.tile_pool(name="pm", bufs=1, space="PSUM"))
        ps = ctx.enter_context(tc.tile_pool(name="ps", bufs=2, space="PSUM"))
        pDO = ctx.enter_context(tc.tile_pool(name="pDO", bufs=1, space="PSUM"))

        # ---- load constants ----
        def load(dram, shape, name, dt=F16):
            t = cp.tile(shape, dt, tag=name)
            nc.sync.dma_start(t[:], dram[:])
            return t

        xT_s = load(xT, [2, BL * N], "xT")
        mw1_s = load(mw1, [2, HID], "mw1")
        brow_s = load(brow, [1, 10 * HID], "brow")
        lnr_s = load(lnr, [HID, 12 * HID], "lnr")
        mw2_s = load(mw2, [HID, HID], "mw2")
        wl_s = load(wl, [HID, L * HID], "wl")
        wr_s = load(wr, [HID, L * HID], "wr")
        ewt_s = load(ewt, [NCAT, L * HID], "ewt")
        cm_s = load(catmask, [NCAT, 5 * N], "cm")
        ab08_s = load(ab08, [HID, L * H], "ab08")
        ab02_s = load(ab02, [HID, L * H], "ab02")
        ab02r_s = load(ab02rep, [HID, L * JQ * H], "ab02r")
        blT_s = load(blT, [HID, L], "blT", F32)
        pw_s = load(pw, [HID, L * HID], "pw")
        ow_s = load(ow, [HID, 1], "ow", F32)
        ob_s = load(ob, [1, 1], "ob", F32)

        ident = cp.tile([128, 128], F32, tag="ident")
        make_identity(nc, ident[:])
        ident16 = cp.tile([128, 128], F16, tag="ident16")
        nc.vector.tensor_copy(ident16, ident)
        ones_r = cp.tile([1, N], F16, tag="ones_r")
        nc.gpsimd.memset(ones_r[:], 1.0)
        ones640 = cp.tile([1, 5 * N], F16, tag="ones640")
        nc.gpsimd.memset(ones640[:], 1.0)
        ones_c = cp.tile([128, 1], F16, tag="ones_c")
        nc.gpsimd.memset(ones_c[:], 1.0)
        zb = cp.tile([128, 1], F32, tag="zb")
        nc.gpsimd.memset(zb[:], 0.0)
        epsb = cp.tile([128, 1], F32, tag="epsb")
        nc.gpsimd.memset(epsb[:], EPS)

        # per-graph persistent: augmented [xl | 1] (ones col survives memset)
        xla = pp.tile([128, BL * H * AUG], F16, tag="xla")
        nc.gpsimd.memset(xla[:], 1.0)

        def lnv(i):  # replicated LN vector slice [128, 128]
            return lnr_s[:, i * HID:(i + 1) * HID]

        h_cur = [None] * BL

        # ======== input MLP ========
        for b in range(BL):
            p1 = pg.tile([128, HID], F32, tag="pg")
            nc.tensor.matmul(p1, xT_s[:, b * N:(b + 1) * N], mw1_s[:], start=True, stop=False)
            nc.tensor.matmul(p1, ones_r[:], brow_s[:, 0:HID], start=False, stop=True)
            h1 = wp.tile([128, HID], F32, tag="h1")
            _ln_free(nc, wp, sp, p1[:], lnv(LN1G), lnv(LN1B), h1[:], "a", zb, epsb)
            h1r = wp.tile([128, HID], F16, tag="h1r")
            nc.vector.tensor_scalar_max(h1r, h1, 0.0)
            ptr = pt.tile([128, 128], F16, tag="ptr")
            nc.tensor.transpose(ptr, h1r[:], ident16[:])
            h1T = wp.tile([128, HID], F16, tag="h1T")
            nc.vector.tensor_copy(h1T, ptr)
            p2 = pg.tile([128, HID], F32, tag="pg")
            nc.tensor.matmul(p2, h1T[:], mw2_s[:], start=True, stop=False)
            nc.tensor.matmul(p2, ones_r[:], brow_s[:, HID:2 * HID], start=False, stop=True)
            hb = hp.tile([128, HID], F16, tag=f"h{b}")
            _ln_free(nc, wp, sp, p2[:], lnv(LN2G), lnv(LN2B), hb[:], "b", zb, epsb)
            h_cur[b] = hb

        # ======== GATv2 layers ========
        # Scores use lrelu = 0.2*m + 0.8*relu(m): the 0.2-linear xl/e part is a
        # per-q broadcast matmul into psb, the xr part is per-(j,h)-constant and
        # dropped (softmax-invariant; dropped consistently in the diag terms).
        # The two graphs are emitted phase-interleaved for pipeline overlap.
        PE_Q = int(os.environ.get("KERNEL_PE_Q", "1"))       # q-blocks on PE
        DVE_CHUNKS = int(os.environ.get("KERNEL_DVE_CHUNKS", "4"))  # PSUM relu chunks on DVE (of 8)

        def prep(l, b):
            wls = wl_s[:, l * HID:(l + 1) * HID]
            wrs = wr_s[:, l * HID:(l + 1) * HID]
            ewtl = ewt_s[:, l * HID:(l + 1) * HID]
            blrow = brow_s[:, (2 + l) * HID:(3 + l) * HID]
            hb = h_cur[b]
            xlab = xla[:, b * H * AUG:(b + 1) * H * AUG]
            ptr = pt.tile([128, 128], F16, tag="ptr")
            nc.tensor.transpose(ptr, hb[:], ident16[:])
            hT16 = wp.tile([128, HID], F16, tag="hT16")
            nc.scalar.activation(hT16, ptr, AF.Identity, bias=zb)

            pxq = pm.tile([128, 4 * 128], F32, tag="pxq")
            nc.tensor.matmul(pxq.rearrange("k (a i) -> k a i", a=4),
                             wls,
                             hT16.rearrange("k (o i) -> k o i", o=1)
                             .broadcast_to((128, 4, 128)), start=True, stop=False)
            nc.tensor.matmul(pxq, ewtl[0:NCAT, :], cm_s[0:NCAT, 0:4 * N],
                             start=False, stop=False)
            nc.tensor.matmul(pxq, blrow, ones640[:, 0:4 * N], start=False, stop=True)
            xlq = wp.tile([128, 4 * 128], F16, tag="xlq")
            nc.scalar.activation(xlq, pxq, AF.Identity, bias=zb)

            pxd = pg.tile([128, HID], F32, tag="pg")
            nc.tensor.matmul(pxd, wls, hT16[:], start=True, stop=False)
            nc.tensor.matmul(pxd, ewtl[0:NCAT, :], cm_s[0:NCAT, 4 * N:5 * N],
                             start=False, stop=False)
            nc.tensor.matmul(pxd, blrow, ones_r[:], start=False, stop=True)
            xld16 = wp.tile([128, HID], F16, tag="xld16")
            nc.scalar.activation(xld16, pxd, AF.Identity, bias=zb)

            pxr = pg.tile([128, HID], F32, tag="pg")
            nc.tensor.matmul(pxr, wrs, hT16[:], start=True, stop=True)
            xrT = wp.tile([128, HID], F16, tag="xrT")
            nc.scalar.activation(xrT, pxr, AF.Identity, bias=zb)
            pxr2 = pg.tile([128, HID], F32, tag="pg")
            nc.tensor.matmul(pxr2, hT16[:], wrs, start=True, stop=True)
            xrn = wp.tile([128, HID], F16, tag="xrn")
            nc.scalar.activation(xrn, pxr2, AF.Identity, bias=zb)

            pxn = pg.tile([128, HID], F32, tag="pg")
            nc.tensor.matmul(pxn, hT16[:], wls, start=True, stop=False)
            nc.tensor.matmul(pxn, ones_r[:], blrow, start=False, stop=True)
            nc.scalar.activation(
                xlab.rearrange("i (h q) -> i h q", q=AUG)[:, :, 0:C],
                pxn.rearrange("i (h c) -> i h c", c=C), AF.Identity, bias=zb)

            md = wp.tile([128, 128], F16, tag="md")
            nc.vector.tensor_add(md, xld16, xrT)
            mdl = wp.tile([128, 128], F16, tag="mdl")
            nc.vector.tensor_scalar_max(mdl, md, 0.0)
            mab = mb.tile([128, N * 128], F16, tag="mab")
            return dict(xlq=xlq, xld16=xld16, xrT=xrT, xrn=xrn, mdl=mdl,
                        mab=mab, xlab=xlab, psbh=[None, None], es=None)

        def mblock(l, b, q, d):
            ab08l = ab08_s[:, l * H:(l + 1) * H]
            ab02rl = ab02r_s[:, l * JQ * H:(l + 1) * JQ * H]
            xlq, xrT, xrn, mab = d["xlq"], d["xrT"], d["xrn"], d["mab"]
            half = q // 2
            if q % 2 == 0:
                psbh_t = ps.tile([128, JQ * 2 * H], F32, tag="psbh")
                d["psbh"][half] = psbh_t
            psbh = d["psbh"][half]
            pcols = (q % 2) * JQ * H
            nc.tensor.matmul(psbh[:, pcols:pcols + JQ * H],
                             xlq[:, q * 128:(q + 1) * 128],
                             ab02rl, start=True, stop=False)
            if q >= NQ - PE_Q:
                for c in range(8):
                    j0 = q * JQ + c * 4
                    pmq = pm.tile([128, 512], F32, tag="pmq")
                    nc.tensor.matmul(
                        pmq.rearrange("k (j i) -> k j i", j=4),
                        ident16[:],
                        xlq[:, q * 128:(q + 1) * 128]
                        .rearrange("k (o i) -> k o i", o=1)
                        .broadcast_to((128, 4, 128)), start=True, stop=False)
                    for jj in range(4):
                        nc.tensor.matmul(
                            pmq[:, jj * 128:(jj + 1) * 128],
                            xrn[:],
                            ident16[:, j0 + jj:j0 + jj + 1]
                            .broadcast_to((128, 128)),
                            start=False, stop=jj == 3,
                            skip_group_check=jj != 3)
                    dst = mab[:, j0 * 128:(j0 + 4) * 128]
                    if c < 8 - DVE_CHUNKS:
                        nc.scalar.activation(dst, pmq, AF.Relu, bias=zb)
                    else:
                        nc.vector.tensor_scalar_max(dst, pmq, 0.0)
            else:
                for hh in range(2):
                    m_q = mb.tile([128, JQ * 64], F16, tag="m_q")
                    j0 = q * JQ + hh * 16
                    nc.vector.scalar_tensor_tensor(
                        m_q.rearrange("k (j i) -> k j i", j=16),
                        xlq[:, q * 128:(q + 1) * 128]
                        .rearrange("k (o i) -> k o i", o=1)
                        .broadcast_to((128, 16, 128)),
                        0.0,
                        xrT[:, j0:j0 + 16]
                        .rearrange("k (j o) -> k j o", o=1)
                        .broadcast_to((128, 16, 128)),
                        op0=OP.add, op1=OP.add)
                    nc.scalar.activation(mab[:, j0 * 128:(j0 + 16) * 128],
                                         m_q[:], AF.Relu, bias=zb)
            for jj in range(JQ):
                j = q * JQ + jj
                last = jj == JQ - 1
                nc.tensor.matmul(psbh[:, pcols + jj * H:pcols + (jj + 1) * H],
                                 mab[:, j * 128:(j + 1) * 128], ab08l,
                                 start=False, stop=last,
                                 skip_group_check=not last)
            if q % 2 == 1:
                if d["es"] is None:
                    es_t = wp.tile([128, N * H], F16, tag="es")
                    d["es"] = es_t
                nc.scalar.activation(d["es"][:, half * JQ * 2 * H:(half + 1) * JQ * 2 * H],
                                     psbh[:], AF.Exp, bias=zb)

        def tail(l, b, d):
            ab08l = ab08_s[:, l * H:(l + 1) * H]
            ab02l = ab02_s[:, l * H:(l + 1) * H]
            pws = pw_s[:, l * HID:(l + 1) * HID]
            xlq, mab, mdl, xlab, esb = d["xlq"], d["mab"], d["mdl"], d["xlab"], d["es"]
            pdo = pDO.tile([128, 160], F32, tag="pdo")
            pd = pdo[:, 144:160]
            po = pdo[:, 0:H * AUG]
            nc.tensor.matmul(pd[:, 0:8], mdl[:], ab08l, start=True, stop=False)
            nc.tensor.matmul(pd[:, 0:8], d["xld16"][:], ab02l, start=False, stop=True)
            dsl = mab[:, 0:(N - 1) * 129 + 1:129]
            nc.tensor.matmul(pd[:, 8:16], dsl, ab08l, start=True, stop=False)
            dlin = wp.tile([128, 128], F16, tag="dlin")
            nc.scalar.activation(
                dlin.rearrange("k (q r) -> k q r", q=NQ),
                xlq.rearrange("k (a b) -> k a b", a=16)[:, 0:16:5, :],
                AF.Identity, bias=zb)
            nc.tensor.matmul(pd[:, 8:16], dlin[:], ab02l, start=False, stop=True)
            ed = sp.tile([128, 16], F32, tag="ed")
            nc.scalar.activation(ed, pd, AF.Exp, bias=zb)
            Dt = sp.tile([128, 8], F32, tag="Dt")
            nc.vector.tensor_sub(Dt, ed[:, 0:8], ed[:, 8:16])

            es3 = esb.rearrange("i (j h) -> i j h", h=H)
            for h in range(H):
                nc.tensor.matmul(po[:, h * AUG:(h + 1) * AUG], es3[:, :, h],
                                 xlab[:, h * AUG:(h + 1) * AUG], start=True, stop=True)
            tcor = wp.tile([128, H * AUG], F32, tag="tcor")
            nc.vector.scalar_tensor_tensor(
                tcor.rearrange("j (h q) -> j h q", q=AUG),
                xlab.rearrange("j (h q) -> j h q", q=AUG), 1.0,
                Dt.rearrange("j (h o) -> j h o", o=1).broadcast_to((128, H, AUG)),
                op0=OP.mult, op1=OP.mult)
            nc.vector.tensor_add(po, po, tcor)

            po3 = po.rearrange("j (h q) -> j h q", q=AUG)
            rz = sp.tile([128, H], F32, tag="rz")
            nc.vector.reciprocal(rz.rearrange("j (h o) -> j h o", o=1),
                                 po3[:, :, C:C + 1])
            o16 = wp.tile([128, HID], F16, tag="o16")
            nc.vector.scalar_tensor_tensor(
                o16.rearrange("j (h c) -> j h c", c=C), po3[:, :, 0:C], 1.0,
                rz.rearrange("j (h o) -> j h o", o=1).broadcast_to((128, H, C)),
                op0=OP.mult, op1=OP.mult)

            pto = pt.tile([128, 128], F16, tag="ptr")
            nc.tensor.transpose(pto, o16[:], ident16[:])
            oT = wp.tile([128, HID], F16, tag="oT")
            nc.scalar.activation(oT, pto, AF.Identity, bias=zb)
            ppj = pg.tile([128, HID], F32, tag="pg")
            nc.tensor.matmul(ppj, oT[:], pws, start=True, stop=False)
            nc.tensor.matmul(ppj, ones_r[:], brow_s[:, (6 + l) * HID:(7 + l) * HID],
                             start=False, stop=True)
            lno = wp.tile([128, HID], F32, tag="lno")
            _ln_free(nc, wp, sp, ppj[:], lnv(LNG0 + l), lnv(LNB0 + l), lno[:], "c", zb, epsb)
            hn = hp.tile([128, HID], F16, tag=f"h{b}")
            nc.vector.scalar_tensor_tensor(hn, lno, 0.0, h_cur[b],
                                           op0=OP.max, op1=OP.add)
            h_cur[b] = hn

        # software-pipeline the two graphs half a layer out of phase: one
        # graph's m-blocks overlap the other's tail+prep dependency chains
        ds = [None, None]
        ds[0] = prep(0, 0)
        for q in range(NQ):
            mblock(0, 0, q, ds[0])
        ds[1] = prep(0, 1)
        for l in range(L):
            for q in range(NQ):
                mblock(l, 1, q, ds[1])
            tail(l, 0, ds[0])
            if l + 1 < L:
                ds[0] = prep(l + 1, 0)
                for q in range(NQ):
                    mblock(l + 1, 0, q, ds[0])
            tail(l, 1, ds[1])
            if l + 1 < L:
                ds[1] = prep(l + 1, 1)

        # ======== pooling + head ========
        for b in range(BL):
            pa = pg.tile([128, 1], F32, tag="pg")
            nc.tensor.matmul(pa, h_cur[b][:], ones_c[:], start=True, stop=True)
            hagg = sp.tile([128, 1], F32, tag="hagg")
            nc.vector.tensor_copy(hagg, pa)
            pr = pg.tile([1, 1], F32, tag="pg")
            nc.tensor.matmul(pr, hagg[:], ow_s[:], start=True, stop=True)
            res = sp.tile([1, 1], F32, tag="res")
            nc.scalar.activation(res, pr, AF.Identity, bias=ob_s[0:1, 0:1])
            nc.sync.dma_start(out[b:b + 1, :], res[:])

    nc.compile()
    return nc


def pack_inputs(inputs):
    """Full model inputs -> per-core in_maps (host-side shard + re-layout)."""
    f = {k: np.asarray(v, dtype=np.float32) if k != "cat" else np.asarray(v)
         for k, v in inputs.items()}
    att = f["att"]
    abk = np.zeros((HID, L * H), np.float32)
    for l in range(L):
        for h in range(H):
            abk[h * C:(h + 1) * C, l * H + h] = att[l, h]

    # category mask [20, 5*128]: block q (4 off-diag target orbits) + diag block
    pos = np.arange(N) // (N // NQ)
    catmask = np.zeros((NCAT, 5 * N), np.float32)
    for q in range(NQ):
        catmask[pos * NQ + q, q * N + np.arange(N)] = 1.0
    catmask[NQ * NQ + pos, NQ * N + np.arange(N)] = 1.0

    pb_eff = np.stack([f["cb"][l] @ f["pW"][l] + f["pb"][l] for l in range(L)])

    # EW: per layer the 20 category embeddings transformed by We [20, HID]
    ewt = np.zeros((NCAT, L * HID), np.float32)
    for l in range(L):
        ewt[:, l * HID:(l + 1) * HID] = f["emb"] @ f["We"][l]

    lnvecs = [f["ln1_g"], f["ln1_b"], f["ln2_g"], f["ln2_b"],
              *[f["lng"][l] for l in range(L)], *[f["lnb"][l] for l in range(L)]]
    lnr = np.ascontiguousarray(
        np.broadcast_to(np.concatenate(lnvecs)[None, :], (HID, 12 * HID))
    ).astype(np.float16)

    def stackw(w):  # [L, k, hc] -> [k, L*hc] so sbuf slice l is W[l][k, hc]
        return np.ascontiguousarray(w.transpose(1, 0, 2).reshape(HID, L * HID))

    shared = {
        "wl": stackw(f["Wl"]).astype(np.float16),
        "wr": stackw(f["Wr"]).astype(np.float16),
        "pw": stackw(f["pW"]).astype(np.float16),
        "ewt": ewt.astype(np.float16),
        "catmask": catmask.astype(np.float16),
        "ab08": (0.8 * abk).astype(np.float16),
        "ab02": (0.2 * abk).astype(np.float16),
        "ab02rep": np.ascontiguousarray(
            np.tile((0.2 * abk).reshape(HID, L, 1, H), (1, 1, JQ, 1))
            .reshape(HID, L * JQ * H)).astype(np.float16),
        "blT": np.ascontiguousarray(f["bl"].T),
        "brow": np.concatenate([f["mlp_b1"], f["mlp_b2"],
                                f["bl"].ravel(), pb_eff.ravel()])
        .reshape(1, 10 * HID).astype(np.float16),
        "mw1": f["mlp_w1"].astype(np.float16),
        "mw2": f["mlp_w2"].astype(np.float16),
        "lnr": lnr, "ow": f["out_w"].reshape(HID, 1),
        "ob": f["out_b"].reshape(1, 1),
    }
    in_maps = []
    for c in range(NCORES):
        xTc = np.ascontiguousarray(
            f["x"][c * BL:(c + 1) * BL].transpose(2, 0, 1)).reshape(2, BL * N)
        m = dict(shared)
        m["xT"] = xTc.astype(np.float16)
        in_maps.append(m)
    return in_maps


_NC = None
LAST_EXEC_NS = None


def kernel(**inputs) -> np.ndarray:
    global _NC, LAST_EXEC_NS
    from concourse.bass_utils import run_bass_kernel_spmd
    if _NC is None:
        _NC = build_nc()
    import os
    in_maps = pack_inputs(inputs)
    trace = bool(os.environ.get("KERNEL_TRACE"))
    kw = {}
    td = os.environ.get("KERNEL_TRACE_DIR")
    if td:
        os.makedirs(td, exist_ok=True)
        kw["tmpdir"] = td
    r = run_bass_kernel_spmd(_NC, in_maps, core_ids=list(range(NCORES)),
                             trace=trace, **kw)
    LAST_EXEC_NS = r.exec_time_ns
    out = np.concatenate([r.results[c]["out"] for c in range(NCORES)], axis=0)
    return out.astype(np.float32)
